# revision 1
# baseline (speedup 1.0000x reference)
"""LocallyConnected1D (B=8, L=4096, C=64, K=3, F=64) on 8 TRN2 NeuronCores.

out[b, l, f] = sum_{k,c} x[b, l+k, c] * kernel[l, k, c, f] + bias[l, f]

Strategy (spatial sharding, 512 output positions per core):
  - For each pair of adjacent output positions (l0+2i, l0+2i+1) build a
    block-diagonal stationary tile lhsT (128 x 16): partitions = 2 phases x 64
    channels, columns = 2 phases x 8 batch.  Streaming operand = the pair's
    per-position weights (128 x 64).  Three PSUM-accumulated matmuls per pair
    (one per tap k, using x-pair tiles shifted by k) produce out (16, 64).
  - Groups of 8 pairs are dispatched to 4 independent 32-column strips of the
    PE array (tile_position), each strip accumulating into its own PSUM bank,
    so up to 4 matmuls run concurrently in the array.
  - Weights AND x-pair tiles are packed into one contiguous DRAM blob per
    block -> dense DMAs at full HBM bandwidth.  First blocks are small so the
    PE starts early; per-block outputs go out in a single DMA.
  - Compute in bf16 (PSUM accumulation in f32); bias added on host.
"""

import numpy as np
import ml_dtypes

import concourse.bass as bass
import concourse.mybir as mybir
import concourse.tile as tile
from concourse import bacc
from concourse.bass import ds, ts
from concourse.bass_utils import run_bass_kernel_spmd

B, L, C, K, F = 8, 4096, 64, 3, 64
L_OUT = (L - K) + 1  # 4094
N_CORES = 8
P_CORE = 512          # output positions per core (last core: 510 real + 2 pad)
PAIRS = P_CORE // 2   # 256

# pairs per DMA block; small first blocks let the PE start early
BLOCKS = [8, 8, 16] + [32] * 6 + [16, 8, 8]
assert sum(BLOCKS) == PAIRS and all(b % 8 == 0 for b in BLOCKS)

USE_BF16 = True
DT = mybir.dt.bfloat16 if USE_BF16 else mybir.dt.float32
NPDT = ml_dtypes.bfloat16 if USE_BF16 else np.float32
DT_OUT = mybir.dt.float32

# per-block columns (per partition): weights | te tiles | to tiles
def _blk_cols(n):
    return n * K * F + (n + 1) * 16 + n * 16

BLK_OFF = np.cumsum([0] + [_blk_cols(n) for n in BLOCKS]).tolist()
TOT_COLS = BLK_OFF[-1]

_CACHE = {}


def _build_body(nc, wpool, opool, pspool, blk_d, out_d):
    s = 0  # first pair of current block
    for h, n in enumerate(BLOCKS):
        cols = _blk_cols(n)
        blk = wpool.tile([128, cols], DT, name="blk", tag="blk",
                         padded_shape=[128, _blk_cols(max(BLOCKS))])
        nc.sync.dma_start(blk[:], blk_d[:, ds(BLK_OFF[h], cols)])
        w_cols = n * K * F
        te_cols = (n + 1) * 16
        ngroups = n // 8
        accs = [pspool.tile([128, 512], DT_OUT, name=f"acc{q}", tag=f"acc{q}")
                for q in range(ngroups)]

        def te_ap(i):   # block-diag tile for even-start pair i
            return blk[:, ds(w_cols + (i - s) * 16, 16)]

        def to_ap(i):   # odd-start pair i
            return blk[:, ds(w_cols + te_cols + (i - s) * 16, 16)]

        def w_ap(jj, k):
            return blk[:, ds((jj * K + k) * F, F)]

        for j in range(8):
            for q in range(ngroups):
                i = s + q * 8 + j   # global pair
                jj = q * 8 + j      # pair in block
                o_ap = accs[q][ds(32 * q, 16), ts(j, 64)]
                tp = (0, 32 * q)
                nc.tensor.matmul(o_ap, te_ap(i), w_ap(jj, 0),
                                 start=True, stop=False, tile_position=tp)
                nc.tensor.matmul(o_ap, to_ap(i), w_ap(jj, 1),
                                 start=False, stop=False, tile_position=tp)
                nc.tensor.matmul(o_ap, te_ap(i + 1), w_ap(jj, 2),
                                 start=False, stop=True, tile_position=tp)
        ob = opool.tile([16, ngroups * 512], DT_OUT, name="ob", tag="ob",
                        padded_shape=[16, 4 * 512])
        for q in range(ngroups):
            nc.vector.tensor_copy(ob[:, ds(q * 512, 512)],
                                  accs[q][ds(32 * q, 16), :])
        g0 = s // 8  # first global group of this block
        nc.scalar.dma_start(out_d[:, ds(g0 * 512, ngroups * 512)], ob[:])
        s += n


def _build_nc(n_iters=None):
    """n_iters=None: straight-line kernel (graded path).
    n_iters=N: body wrapped in a HW For_i loop, for timing-slope runs."""
    nc = bacc.Bacc("TRN2", target_bir_lowering=False, debug=False)

    blk_d = nc.declare_dram_parameter("blk", [128, TOT_COLS], DT, isOutput=False)
    # out[m, g*512 + j*64 + f]: g = group of 8 pairs, m = phase*8 + b.
    out_d = nc.declare_dram_parameter("out", [16, (PAIRS // 8) * 512], DT_OUT,
                                      isOutput=True)

    with tile.TileContext(nc) as tc:
        with (
            tc.tile_pool(name="wpool", bufs=8) as wpool,
            tc.tile_pool(name="opool", bufs=8) as opool,
            # 4 acc tags (one per PE strip) x 2 bufs = all 8 PSUM banks
            tc.tile_pool(name="pspool", bufs=2, space=bass.MemorySpace.PSUM) as pspool,
        ):
            if n_iters is None:
                _build_body(nc, wpool, opool, pspool, blk_d, out_d)
            else:
                with tc.For_i(0, n_iters, 1):
                    _build_body(nc, wpool, opool, pspool, blk_d, out_d)

    nc.compile()
    return nc


def _prep_inputs(x, kernel):
    """Host-side rearrangement into per-core fused block layouts."""
    xp = np.zeros((B, L + 4, C), np.float32)
    xp[:, :L] = x
    kp = np.zeros((N_CORES * P_CORE, K, C, F), np.float32)
    kp[:L_OUT] = kernel
    in_maps = []
    for m in range(N_CORES):
        l0 = P_CORE * m
        xs = xp[:, l0:l0 + 2 * PAIRS + 2, :]
        ev = xs[:, 0::2].transpose(2, 1, 0)  # (64, 257, 8)  j = 2i
        od = xs[:, 1::2].transpose(2, 1, 0)  # (64, 257, 8)  j = 2i+1
        # TE[i]: pair (2i, 2i+1); TO[i]: pair (2i+1, 2i+2); block-diag (128,16)
        TE = np.zeros((128, PAIRS + 1, 16), np.float32)
        TE[:64, :, 0:8] = ev
        TE[64:, :, 8:16] = od
        TO = np.zeros((128, PAIRS, 16), np.float32)
        TO[:64, :, 0:8] = od[:, :PAIRS]
        TO[64:, :, 8:16] = ev[:, 1:PAIRS + 1]
        W = (kp[l0:l0 + P_CORE]
             .reshape(PAIRS, 2, K, C, F)
             .transpose(1, 3, 0, 2, 4)
             .reshape(128, PAIRS, K, F))  # [pc, pair, k, f]
        blk = np.empty((128, TOT_COLS), np.float32)
        s = 0
        for h, n in enumerate(BLOCKS):
            o = BLK_OFF[h]
            w_cols = n * K * F
            blk[:, o:o + w_cols] = W[:, s:s + n].reshape(128, w_cols)
            blk[:, o + w_cols:o + w_cols + (n + 1) * 16] = (
                TE[:, s:s + n + 1].reshape(128, (n + 1) * 16))
            blk[:, o + w_cols + (n + 1) * 16:o + _blk_cols(n)] = (
                TO[:, s:s + n].reshape(128, n * 16))
            s += n
        in_maps.append({"blk": blk.astype(NPDT)})
    return in_maps


def _unpack_out(res):
    """(16, 32*512) per core -> (B, P_CORE, F).  l_local = 16g + 2j + phase."""
    return (res.reshape(2, 8, 32, 8, 64)          # [phase, b, g, j, f]
            .transpose(1, 2, 3, 0, 4)              # [b, g, j, phase, f]
            .reshape(B, P_CORE, F))


def kernel(x, kernel, bias):
    x = np.asarray(x, dtype=np.float32)
    kern = np.asarray(kernel, dtype=np.float32)
    bias = np.asarray(bias, dtype=np.float32)

    if "nc" not in _CACHE:
        _CACHE["nc"] = _build_nc()
    nc = _CACHE["nc"]

    in_maps = _prep_inputs(x, kern)
    results = run_bass_kernel_spmd(nc, in_maps, list(range(N_CORES))).results

    parts = [_unpack_out(results[m]["out"]) for m in range(N_CORES)]
    out = np.concatenate(parts, axis=1)[:, :L_OUT]
    return (out + bias[None]).astype(np.float32)



# revision 2
# speedup vs baseline: 1.1755x; 1.1755x over previous
"""LocallyConnected1D (B=8, L=4096, C=64, K=3, F=64) on 8 TRN2 NeuronCores.

out[b, l, f] = sum_{k,c} x[b, l+k, c] * kernel[l, k, c, f] + bias[l, f]

Strategy (spatial sharding, 512 output positions per core):
  - For each pair of adjacent output positions (l0+2i, l0+2i+1) build a
    block-diagonal stationary tile lhsT (128 x 16): partitions = 2 phases x 64
    channels, columns = 2 phases x 8 batch.  Streaming operand = the pair's
    per-position weights (128 x 64).  Three PSUM-accumulated matmuls per pair
    (one per tap k, using x-pair tiles shifted by k) produce out (16, 64).
  - Weights are stored in fp8-e3m4 (x16 scale folded into x on the host:
    x/16 in bf16) -- halves weight HBM traffic vs bf16 at rel-err ~1.4e-2.
  - Groups of 8 pairs go to 4 independent 32-column strips of the PE array
    (tile_position); all 4 strips of a 32-pair block accumulate into ONE
    PSUM bank at partition offsets 0/32/64/96, so the whole block drains
    with a single 112-partition vector copy (f32 -> bf16) per block.
  - Per block: one fp8 weight DMA + one bf16 x-tile DMA in, ngroups output
    DMAs (16x512 bf16) out.  First blocks are small so the PE starts early.
"""

import numpy as np
import ml_dtypes

import concourse.bass as bass
import concourse.mybir as mybir
import concourse.tile as tile
from concourse import bacc
from concourse.bass import ds, ts
from concourse.bass_utils import run_bass_kernel_spmd

B, L, C, K, F = 8, 4096, 64, 3, 64
L_OUT = (L - K) + 1  # 4094
N_CORES = 8
P_CORE = 512          # output positions per core (last core: 510 real + 2 pad)
PAIRS = P_CORE // 2   # 256

# pairs per DMA block; small first blocks let the PE start early
BLOCKS = [8, 8, 16] + [32] * 6 + [16, 8, 8]
assert sum(BLOCKS) == PAIRS and all(b % 8 == 0 for b in BLOCKS)

W_DT = mybir.dt.float8e3
W_NP = ml_dtypes.float8_e3m4
X_DT = mybir.dt.bfloat16
X_NP = ml_dtypes.bfloat16
O_DT = mybir.dt.bfloat16
O_NP = ml_dtypes.bfloat16
W_SCALE = 16.0  # w stored as e3m4(16*w); x stored as bf16(x/16)

def _w_cols(n):
    return n * K * F

def _x_cols(n):
    return (2 * n + 1) * 16

W_OFF = np.cumsum([0] + [_w_cols(n) for n in BLOCKS]).tolist()
X_OFF = np.cumsum([0] + [_x_cols(n) for n in BLOCKS]).tolist()
W_TOT = W_OFF[-1]
X_TOT = X_OFF[-1]

_CACHE = {}


def _build_body(nc, wpool, xpool, opool, pspool, w_d, x_d, out_d,
                skip_mm=False, skip_in_dma=False):
    s = 0  # first pair of current block
    for h, n in enumerate(BLOCKS):
        wblk = wpool.tile([128, _w_cols(n)], W_DT, name="wblk", tag="w",
                          padded_shape=[128, _w_cols(max(BLOCKS))])
        xblk = xpool.tile([128, _x_cols(n)], X_DT, name="xblk", tag="x",
                          padded_shape=[128, _x_cols(max(BLOCKS))])
        if not skip_in_dma:
            nc.sync.dma_start(wblk[:], w_d[:, ds(W_OFF[h], _w_cols(n))])
            nc.sync.dma_start(xblk[:], x_d[:, ds(X_OFF[h], _x_cols(n))])
        ngroups = n // 8
        p_hi = 32 * (ngroups - 1) + 16  # highest used PSUM partition + 1
        acc = pspool.tile([128, 512], mybir.dt.float32, name="acc", tag="acc")

        def te_ap(i):   # block-diag x tile for even-start pair i (in block)
            return xblk[:, ds(i * 16, 16)]

        def to_ap(i):   # odd-start pair i (in block)
            return xblk[:, ds((n + 1 + i) * 16, 16)]

        def w_ap(i, k):
            return wblk[:, ds((i * K + k) * F, F)]

        if not skip_mm:
            for j in range(8):
                for q in range(ngroups):
                    i = q * 8 + j   # pair in block
                    o_ap = acc[ds(32 * q, 16), ts(j, 64)]
                    tp = (0, 32 * q)
                    nc.tensor.matmul(o_ap, te_ap(i), w_ap(i, 0),
                                     start=True, stop=False, tile_position=tp)
                    nc.tensor.matmul(o_ap, to_ap(i), w_ap(i, 1),
                                     start=False, stop=False, tile_position=tp)
                    nc.tensor.matmul(o_ap, te_ap(i + 1), w_ap(i, 2),
                                     start=False, stop=True, tile_position=tp)
        ob = opool.tile([128, 512], O_DT, name="ob", tag="ob")
        nc.vector.tensor_copy(ob[ds(0, p_hi), :], acc[ds(0, p_hi), :])
        g0 = s // 8  # first global group of this block
        for q in range(ngroups):
            nc.scalar.dma_start(out_d[:, ds((g0 + q) * 512, 512)],
                                ob[ds(32 * q, 16), :])
        s += n


def _build_nc(n_iters=None, skip_mm=False, skip_in_dma=False):
    """n_iters=None: straight-line kernel (graded path).
    n_iters=N: body wrapped in a HW For_i loop, for timing-slope runs."""
    nc = bacc.Bacc("TRN2", target_bir_lowering=False, debug=False)

    w_d = nc.declare_dram_parameter("wb", [128, W_TOT], W_DT, isOutput=False)
    x_d = nc.declare_dram_parameter("xb", [128, X_TOT], X_DT, isOutput=False)
    # out[m, g*512 + j*64 + f]: g = group of 8 pairs, m = phase*8 + b.
    out_d = nc.declare_dram_parameter("out", [16, (PAIRS // 8) * 512], O_DT,
                                      isOutput=True)

    with tile.TileContext(nc) as tc:
        with (
            tc.tile_pool(name="wpool", bufs=6) as wpool,
            tc.tile_pool(name="xpool", bufs=6) as xpool,
            tc.tile_pool(name="opool", bufs=4) as opool,
            tc.tile_pool(name="pspool", bufs=8, space=bass.MemorySpace.PSUM) as pspool,
        ):
            if n_iters is None:
                _build_body(nc, wpool, xpool, opool, pspool, w_d, x_d, out_d,
                            skip_mm=skip_mm, skip_in_dma=skip_in_dma)
            else:
                with tc.For_i(0, n_iters, 1):
                    _build_body(nc, wpool, xpool, opool, pspool, w_d, x_d,
                                out_d, skip_mm=skip_mm, skip_in_dma=skip_in_dma)

    nc.compile()
    return nc


def _prep_inputs(x, kernel):
    """Host-side rearrangement into per-core w/x blobs."""
    xp = np.zeros((B, L + 4, C), np.float32)
    xp[:, :L] = x * (1.0 / W_SCALE)
    kp = np.zeros((N_CORES * P_CORE, K, C, F), np.float32)
    kp[:L_OUT] = kernel * W_SCALE
    in_maps = []
    for m in range(N_CORES):
        l0 = P_CORE * m
        xs = xp[:, l0:l0 + 2 * PAIRS + 2, :]
        ev = xs[:, 0::2].transpose(2, 1, 0)  # (64, 257, 8)  j = 2i
        od = xs[:, 1::2].transpose(2, 1, 0)  # (64, 257, 8)  j = 2i+1
        # TE[i]: pair (2i, 2i+1); TO[i]: pair (2i+1, 2i+2); block-diag (128,16)
        TE = np.zeros((128, PAIRS + 1, 16), np.float32)
        TE[:64, :, 0:8] = ev
        TE[64:, :, 8:16] = od
        TO = np.zeros((128, PAIRS, 16), np.float32)
        TO[:64, :, 0:8] = od[:, :PAIRS]
        TO[64:, :, 8:16] = ev[:, 1:PAIRS + 1]
        W = (kp[l0:l0 + P_CORE]
             .reshape(PAIRS, 2, K, C, F)
             .transpose(1, 3, 0, 2, 4)
             .reshape(128, PAIRS, K, F))  # [pc, pair, k, f]
        wblob = np.empty((128, W_TOT), np.float32)
        xblob = np.empty((128, X_TOT), np.float32)
        sblk = 0
        for h, n in enumerate(BLOCKS):
            wblob[:, W_OFF[h]:W_OFF[h + 1]] = (
                W[:, sblk:sblk + n].reshape(128, _w_cols(n)))
            xo = X_OFF[h]
            xblob[:, xo:xo + (n + 1) * 16] = (
                TE[:, sblk:sblk + n + 1].reshape(128, (n + 1) * 16))
            xblob[:, xo + (n + 1) * 16:X_OFF[h + 1]] = (
                TO[:, sblk:sblk + n].reshape(128, n * 16))
            sblk += n
        in_maps.append({"wb": wblob.astype(W_NP), "xb": xblob.astype(X_NP)})
    return in_maps


def _unpack_out(res):
    """(16, 32*512) per core -> (B, P_CORE, F).  l_local = 16g + 2j + phase."""
    return (np.asarray(res, np.float32)
            .reshape(2, 8, 32, 8, 64)              # [phase, b, g, j, f]
            .transpose(1, 2, 3, 0, 4)              # [b, g, j, phase, f]
            .reshape(B, P_CORE, F))


def kernel(x, kernel, bias):
    x = np.asarray(x, dtype=np.float32)
    kern = np.asarray(kernel, dtype=np.float32)
    bias = np.asarray(bias, dtype=np.float32)

    if "nc" not in _CACHE:
        _CACHE["nc"] = _build_nc()
    nc = _CACHE["nc"]

    in_maps = _prep_inputs(x, kern)
    results = run_bass_kernel_spmd(nc, in_maps, list(range(N_CORES))).results

    parts = [_unpack_out(results[m]["out"]) for m in range(N_CORES)]
    out = np.concatenate(parts, axis=1)[:, :L_OUT]
    return (out + bias[None]).astype(np.float32)


# revision 8
# speedup vs baseline: 1.4291x; 1.2158x over previous
"""LocallyConnected1D (B=8, L=4096, C=64, K=3, F=64) on 8 TRN2 NeuronCores.

out[b, l, f] = sum_{k,c} x[b, l+k, c] * kernel[l, k, c, f] + bias[l, f]

Strategy (spatial sharding, 512 output positions per core):
  - For each pair of adjacent output positions (l0+2i, l0+2i+1) build a
    block-diagonal stationary tile lhsT (128 x 16): partitions = 2 phases x 64
    channels, columns = 2 phases x 8 batch.  Streaming operand = the pair's
    per-position weights; PSUM accumulates the K=3 taps per pair.
  - Weights are stored in fp8-e3m4 (x16 scale folded into x on the host:
    x/16 in bf16) -- halves weight HBM traffic vs bf16 at rel-err ~1.4e-2.
  - One fused 1-byte DMA blob per block: [w fp8 | x tiles as bf16 bytes];
    the x region is bitcast to bf16 on-chip.  Few large DMAs -> full HBM bw.
  - Matmul chain merge: pair i's tap-2 and pair i+1's tap-0 share the same
    stationary x tile and are column-adjacent in the blob, so they issue as
    ONE 128-column matmul.  PSUM banks are pre-zeroed (scalar-engine memset)
    so every matmul runs start=False and merges freely.  17 MMs per group of
    8 pairs instead of 24.
  - A block's 2-4 groups go to separate 32-col PE strips (tile_position),
    all accumulating into ONE PSUM bank at partition offsets 0/32/64/96; the
    block drains with a single 112-partition vector copy (f32 -> bf16) into
    a resident SBUF output buffer, which is flushed to HBM in 3 large DMAs.
"""

import numpy as np
import ml_dtypes

import concourse.bass as bass
import concourse.mybir as mybir
import concourse.tile as tile
from concourse import bacc
from concourse.bass import ds, ts
from concourse.bass_utils import run_bass_kernel_spmd

B, L, C, K, F = 8, 4096, 64, 3, 64
L_OUT = (L - K) + 1  # 4094
N_CORES = 8
P_CORE = 512          # output positions per core (last core: 510 real + 2 pad)
PAIRS = P_CORE // 2   # 256

# pairs per DMA block; small first blocks let the PE start early
BLOCKS = [16, 16] + [32] * 7
assert sum(BLOCKS) == PAIRS and all(b % 8 == 0 for b in BLOCKS)
N_BLOCKS = len(BLOCKS)
G0 = np.cumsum([0] + [n // 8 for n in BLOCKS]).tolist()  # first group per blk

W_NP = ml_dtypes.float8_e3m4
X_NP = ml_dtypes.bfloat16
BLOB_DT = mybir.dt.float8e3
X_DT = mybir.dt.bfloat16
O_DT = mybir.dt.bfloat16
W_SCALE = 16.0  # w stored as e3m4(16*w); x stored as bf16(x/16)

def _w_bytes(n):
    return n * K * F          # fp8: 1 byte each; [chain 1024 | k1 512] per grp

def _x_bytes(n):
    return (2 * n + 1) * 16 * 2   # bf16 TE/TO tiles

def _blk_bytes(n):
    return _w_bytes(n) + _x_bytes(n)

OFF = np.cumsum([0] + [_blk_bytes(n) for n in BLOCKS]).tolist()
TOT = OFF[-1]

# output slots: one 512-col slot per block in the resident SBUF buffer
OUT_COLS = 512 * N_BLOCKS
# flush after these blocks (3 large DMAs)
FLUSH = {2: (0, 3), 5: (3, 6), 8: (6, 9)}

_CACHE = {}


def _build_body(nc, pools, blob_d, out_d, variant="full", static_tiles=None):
    bpool, opool, pspool = pools
    do_mm = variant in ("full", "mm", "nooutpath", "mmonly")
    do_in_dma = variant in ("full", "dma", "nooutpath", "indma")
    do_outpath = variant in ("full", "dma", "mm", "outpath")

    ob = opool.tile([128, OUT_COLS], O_DT, name="ob", tag="ob")
    s = 0  # first pair of current block
    for h, n in enumerate(BLOCKS):
        if static_tiles is not None:
            blk = static_tiles[h]
        else:
            blk = bpool.tile([128, _blk_bytes(n)], BLOB_DT, name="blk",
                             tag="blk", padded_shape=[128, _blk_bytes(32)])
            if do_in_dma:
                nc.sync.dma_start(blk[:], blob_d[:, ds(OFF[h], _blk_bytes(n))])
        ngroups = n // 8
        p_hi = 32 * (ngroups - 1) + 16  # highest used PSUM partition + 1
        xv = blk[:, ds(_w_bytes(n), _x_bytes(n))].bitcast(X_DT)

        def te_ap(i):   # block-diag x tile for even-start pair i (in block)
            return xv[:, ds(i * 16, 16)]

        def to_ap(i):   # odd-start pair i (in block)
            return xv[:, ds((n + 1 + i) * 16, 16)]

        def w_chain(q, c0, w):  # chain cols [c0, c0+w) of group q
            return blk[:, ds(q * 1536 + c0 * 64, w * 64)]

        def w_k1(q, i):
            return blk[:, ds(q * 1536 + 1024 + i * 64, 64)]

        if do_mm:
            acc = pspool.tile([128, 512], mybir.dt.float32, name="acc",
                              tag="acc")
            nc.scalar.memzero(acc[ds(0, p_hi), :])
            for step in range(2 * 8 + 1):
                for q in range(ngroups):
                    i = step // 2  # pair within group
                    ii = q * 8 + i  # pair within block (x-tile index)
                    if step % 2 == 0:  # te(i): merged [p_{i-1}k2 | p_i k0]
                        if i == 0:
                            o_ap = acc[ds(32 * q, 16), ds(0, 64)]
                            w_ap = w_chain(q, 0, 1)
                        elif i == 8:
                            o_ap = acc[ds(32 * q, 16), ds(7 * 64, 64)]
                            w_ap = w_chain(q, 15, 1)
                        else:
                            o_ap = acc[ds(32 * q, 16), ds((i - 1) * 64, 128)]
                            w_ap = w_chain(q, 2 * i - 1, 2)
                        x_ap = te_ap(ii)
                    else:              # to(i): k1 tap of pair i
                        o_ap = acc[ds(32 * q, 16), ds(i * 64, 64)]
                        w_ap = w_k1(q, i)
                        x_ap = to_ap(ii)
                    nc.tensor.matmul(o_ap, x_ap, w_ap, start=False,
                                     stop=False, tile_position=(0, 32 * q),
                                     skip_group_check=True)
        if do_outpath:
            if do_mm:
                nc.vector.tensor_copy(ob[ds(0, p_hi), ts(h, 512)],
                                      acc[ds(0, p_hi), :])
            else:
                cw = min(512, (2 * n + 1) * 16)
                nc.vector.tensor_copy(ob[ds(0, p_hi), ds(h * 512, cw)],
                                      xv[ds(0, p_hi), ds(0, cw)])
            if h in FLUSH:
                a, b = FLUSH[h]
                nc.scalar.dma_start(out_d[:, ds(a * 512, (b - a) * 512)],
                                    ob[:, ds(a * 512, (b - a) * 512)])
        s += n


def _build_nc(n_iters=None, variant="full"):
    """n_iters=None: straight-line kernel (graded path).
    n_iters=N: body wrapped in a HW For_i loop, for timing-slope runs."""
    nc = bacc.Bacc("TRN2", target_bir_lowering=False, debug=False)

    blob_d = nc.declare_dram_parameter("blob", [128, TOT], BLOB_DT,
                                       isOutput=False)
    # out[p, h*512 + j*64 + f]: p = 32q + phase*8 + b; block h, strip q
    out_d = nc.declare_dram_parameter("out", [128, OUT_COLS], O_DT,
                                      isOutput=True)

    with tile.TileContext(nc) as tc:
        with (
            tc.tile_pool(name="bpool", bufs=5) as bpool,
            tc.tile_pool(name="opool", bufs=2) as opool,
            tc.tile_pool(name="spool", bufs=N_BLOCKS) as spool,
            tc.tile_pool(name="pspool", bufs=8, space=bass.MemorySpace.PSUM) as pspool,
        ):
            pools = (bpool, opool, pspool)
            static_tiles = None
            if variant in ("mm", "mmonly", "outpath"):
                static_tiles = []
                for h, n in enumerate(BLOCKS):
                    blk = spool.tile([128, _blk_bytes(n)], BLOB_DT,
                                     name=f"sblk{h}", tag="sblk")
                    nc.sync.dma_start(blk[:], blob_d[:, ds(OFF[h], _blk_bytes(n))])
                    static_tiles.append(blk)
            if n_iters is None:
                _build_body(nc, pools, blob_d, out_d, variant=variant,
                            static_tiles=static_tiles)
            else:
                with tc.For_i(0, n_iters, 1):
                    _build_body(nc, pools, blob_d, out_d, variant=variant,
                                static_tiles=static_tiles)

    nc.compile()
    return nc


def _prep_inputs(x, kernel):
    """Host-side rearrangement into per-core fused byte blobs."""
    xp = np.zeros((B, L + 4, C), np.float32)
    xp[:, :L] = x * (1.0 / W_SCALE)
    kp = np.zeros((N_CORES * P_CORE, K, C, F), np.float32)
    kp[:L_OUT] = kernel * W_SCALE
    in_maps = []
    for m in range(N_CORES):
        l0 = P_CORE * m
        xs = xp[:, l0:l0 + 2 * PAIRS + 2, :]
        ev = xs[:, 0::2].transpose(2, 1, 0)  # (64, 257, 8)  j = 2i
        od = xs[:, 1::2].transpose(2, 1, 0)  # (64, 257, 8)  j = 2i+1
        # TE[i]: pair (2i, 2i+1); TO[i]: pair (2i+1, 2i+2); block-diag (128,16)
        TE = np.zeros((128, PAIRS + 1, 16), np.float32)
        TE[:64, :, 0:8] = ev
        TE[64:, :, 8:16] = od
        TO = np.zeros((128, PAIRS, 16), np.float32)
        TO[:64, :, 0:8] = od[:, :PAIRS]
        TO[64:, :, 8:16] = ev[:, 1:PAIRS + 1]
        W = (kp[l0:l0 + P_CORE]
             .reshape(PAIRS, 2, K, C, F)
             .transpose(1, 3, 0, 2, 4)
             .reshape(128, PAIRS, K, F))  # [pc, pair, k, f]
        Wq = W.astype(W_NP)
        blob = np.empty((128, TOT), np.uint8)
        sblk = 0
        for h, n in enumerate(BLOCKS):
            o = OFF[h]
            for q in range(n // 8):
                p0 = sblk + 8 * q
                gw = np.empty((128, 24, 64), W_NP)  # [chain(16) | k1(8)]
                gw[:, 0:16:2] = Wq[:, p0:p0 + 8, 0]
                gw[:, 1:16:2] = Wq[:, p0:p0 + 8, 2]
                gw[:, 16:24] = Wq[:, p0:p0 + 8, 1]
                blob[:, o + q * 1536:o + (q + 1) * 1536] = (
                    gw.reshape(128, 1536).view(np.uint8))
            xo = o + _w_bytes(n)
            te_b = TE[:, sblk:sblk + n + 1].astype(X_NP)
            to_b = TO[:, sblk:sblk + n].astype(X_NP)
            blob[:, xo:xo + (n + 1) * 32] = (
                te_b.reshape(128, -1).view(np.uint8).reshape(128, -1))
            blob[:, xo + (n + 1) * 32:OFF[h + 1]] = (
                to_b.reshape(128, -1).view(np.uint8).reshape(128, -1))
            sblk += n
        in_maps.append({"blob": blob.view(W_NP)})
    return in_maps


def _unpack_out(res):
    """(128, 512*N_BLOCKS) per core -> (B, P_CORE, F)."""
    r = np.asarray(res, np.float32)
    out = np.empty((B, P_CORE, F), np.float32)
    for h, n in enumerate(BLOCKS):
        for q in range(n // 8):
            g = G0[h] + q
            # rows 32q + phase*8 + b; cols h*512 + j*64 + f
            band = r[32 * q:32 * q + 16, 512 * h:512 * (h + 1)]
            band = band.reshape(2, 8, 8, 64)        # [phase, b, j, f]
            l0 = g * 16
            out[:, l0 + 0:l0 + 16:2] = band[0].transpose(0, 1, 2)
            out[:, l0 + 1:l0 + 16:2] = band[1]
    return out


def kernel(x, kernel, bias):
    x = np.asarray(x, dtype=np.float32)
    kern = np.asarray(kernel, dtype=np.float32)
    bias = np.asarray(bias, dtype=np.float32)

    if "nc" not in _CACHE:
        _CACHE["nc"] = _build_nc()
    nc = _CACHE["nc"]

    in_maps = _prep_inputs(x, kern)
    results = run_bass_kernel_spmd(nc, in_maps, list(range(N_CORES))).results

    parts = [_unpack_out(results[m]["out"]) for m in range(N_CORES)]
    out = np.concatenate(parts, axis=1)[:, :L_OUT]
    return (out + bias[None]).astype(np.float32)
